# revision 10
# baseline (speedup 1.0000x reference)
"""Distributed Trainium2 kernel for nn_AttentionMechanism (GNN message passing).

Reference math (single head GAT-style attention over N=500000 neighbors):
    nv   = n_x.reshape(64, N)          (raw row-major reshape, NOT a transpose)
    e    = a[:64].T @ (W@x) * 1s + (a[64:].T @ W) @ nv      (1, N)
    alpha= softmax(e)                   (softmax is shift-invariant -> the
                                         a[:64].(W@x) term cancels)
    out  = sigmoid(W @ (alpha @ n_x).T)

So with c = a[64:,0] @ W (a length-64 vector), alpha = softmax(c @ nv).
Energies are ~N(0,1) so exp() without max-subtraction is numerically safe.

Sharding: neighbor dim split across 8 cores (62500 each). Each core
computes its energies e_j = sum_i c_i * nv[i, j] (VectorE FMA over the
64 i-terms), u = exp(e) (ScalarE, with accum_out giving sum(u) free),
and the partial aggregate pagg = sum_j u_j * n_x[j, :] (TensorE,
contracting j over partitions in groups of 128). Host combines the 8
partial (pagg, sum_u) pairs, normalizes, applies W and sigmoid.
No device collectives needed.
"""

import numpy as np

from concourse import bacc, bass, bass2jax, mybir, tile

N_CORES = 8
SIZE_IN = 64
N_NEIGH = 500000
SHARD = N_NEIGH // N_CORES  # 62500
P = 125                     # used partitions (125 * 500 = 62500)
R = 500                     # neighbor rows per partition
NI = SIZE_IN                # 64
ICH = 8                     # i-values per A chunk (8 chunks of 8)
G = 5                       # r-columns per B matmul (rhs free = 5*64=320)
NB = R // G                 # 100 B matmuls
BCH = 10                    # B dma chunks
RB = R // BCH               # 50 rows per B dma chunk

F32 = mybir.dt.float32


def build_nc(n_devices=N_CORES):
    nc = bacc.Bacc(
        "TRN2",
        target_bir_lowering=False,
        debug=False,
        num_devices=n_devices,
    )
    a_ext = nc.dram_tensor("ablk", [128, NI, R], F32, kind="ExternalInput")
    b_ext = nc.dram_tensor("bblk", [128, R, NI], F32, kind="ExternalInput")
    c_ext = nc.dram_tensor("cvec", [128, NI], F32, kind="ExternalInput")
    pout_ext = nc.dram_tensor("pout", [G, G * NI], F32, kind="ExternalOutput")
    usum_ext = nc.dram_tensor("usum", [P, 1], F32, kind="ExternalOutput")

    with tile.TileContext(nc) as tc:
        with (
            tc.tile_pool(name="apool", bufs=2) as apool,
            tc.tile_pool(name="bpool", bufs=1) as bpool,
            tc.tile_pool(name="spool", bufs=1) as spool,
            tc.tile_pool(name="psum", bufs=1, space=bass.MemorySpace.PSUM) as pp,
        ):
            ctile = spool.tile([128, NI], F32)
            acc = spool.tile([128, R], F32)
            u = spool.tile([128, R], F32)
            usum_sb = spool.tile([128, 1], F32)
            pout_sb = spool.tile([G, G * NI], F32)
            bsb = bpool.tile([128, R, NI], F32)

            nc.sync.dma_start(ctile[:], c_ext[:])
            nc.vector.memset(acc[:], 0.0)
            nc.vector.memset(u[:], 0.0)

            # ---- A phase: energies e[p, r] = sum_i c_i * ablk[p, i, r] ----
            for ci in range(NI // ICH):
                at = apool.tile([128, ICH, R], F32, tag="achunk")
                nc.sync.dma_start(
                    at[:], a_ext[:, ci * ICH : (ci + 1) * ICH, :]
                )
                for il in range(ICH):
                    i = ci * ICH + il
                    nc.vector.scalar_tensor_tensor(
                        out=acc[:],
                        in0=at[:, il, :],
                        scalar=ctile[:, i : i + 1],
                        in1=acc[:],
                        op0=mybir.AluOpType.mult,
                        op1=mybir.AluOpType.add,
                    )

            # ---- B dma: full-resident neighbor rows ----
            for cb in range(BCH):
                nc.sync.dma_start(
                    bsb[:, cb * RB : (cb + 1) * RB, :],
                    b_ext[:, cb * RB : (cb + 1) * RB, :],
                )

            # ---- exp + per-partition sums ----
            nc.scalar.activation(
                u[0:P, :],
                acc[0:P, :],
                mybir.ActivationFunctionType.Exp,
                accum_out=usum_sb[0:P, :],
            )

            # ---- B phase: pagg via PSUM-accumulated matmuls ----
            pacc = pp.tile([G, G * NI], F32)
            for T in range(NB):
                nc.tensor.matmul(
                    pacc[:],
                    u[:, G * T : G * (T + 1)],
                    bsb[:, G * T : G * (T + 1), :],
                    start=(T == 0),
                    stop=(T == NB - 1),
                )
            nc.vector.tensor_copy(pout_sb[:], pacc[:])

            nc.sync.dma_start(pout_ext[:], pout_sb[:])
            nc.sync.dma_start(usum_ext[:], usum_sb[0:P, :])

    nc.compile()
    return nc


_NC = None
_JITTED = None
_IO = None


def _get_nc():
    global _NC
    if _NC is None:
        _NC = build_nc()
    return _NC


def _get_runner():
    """Per-device async launcher.

    The stock ``run_bass_kernel_spmd`` axon path uses an 8-device
    ``shard_map``, which hangs in this container. Instead we jit the
    ``bass_exec`` primitive once and dispatch it on each NeuronCore
    with that core's shard (device-committed inputs), gathering all 8
    results afterwards — same NEFF, same per-core shapes, fully
    concurrent dispatch.
    """
    global _JITTED, _IO
    if _JITTED is not None:
        return _JITTED, _IO

    import jax

    nc = _get_nc()
    bass2jax.install_neuronx_cc_hook()
    assert nc.dbg_addr is None

    in_names = []
    out_names = []
    out_avals = []
    zero_outs = []
    for alloc in nc.m.functions[0].allocations:
        if not isinstance(alloc, mybir.MemoryLocationSet):
            continue
        name = alloc.memorylocations[0].name
        if alloc.kind == "ExternalInput":
            in_names.append(name)
        elif alloc.kind == "ExternalOutput":
            shape = tuple(alloc.tensor_shape)
            dtype = mybir.dt.np(alloc.dtype)
            out_names.append(name)
            out_avals.append(jax.core.ShapedArray(shape, dtype))
            zero_outs.append(np.zeros(shape, dtype))

    n_params = len(in_names)
    all_names = in_names + out_names
    donate = tuple(range(n_params, n_params + len(out_names)))

    def _body(*args):
        outs = bass2jax._bass_exec_p.bind(
            *args,
            out_avals=tuple(out_avals),
            in_names=tuple(all_names),
            out_names=tuple(out_names),
            lowering_input_output_aliases=(),
            sim_require_finite=True,
            sim_require_nnan=True,
            nc=nc,
        )
        return tuple(outs)

    _JITTED = jax.jit(_body, donate_argnums=donate, keep_unused=True)
    _IO = (in_names, out_names, zero_outs)
    return _JITTED, _IO


def _run_per_device(in_maps):
    import time

    import jax

    jitted, (in_names, out_names, zero_outs) = _get_runner()
    devs = jax.devices()[: len(in_maps)]

    def _dispatch(k):
        im = {**in_maps[k], "partition_id": np.array([[k]], np.uint32)}
        args = [jax.device_put(np.asarray(im[n]), devs[k]) for n in in_names]
        zs = [jax.device_put(z, devs[k]) for z in zero_outs]
        return jitted(*args, *zs)

    results = [None] * len(in_maps)
    pending = list(range(len(in_maps)))
    last_err = None
    for attempt in range(5):
        futs = {}
        failed = []
        for k in pending:
            try:
                futs[k] = _dispatch(k)
            except Exception as e:  # transient device errors; retry
                last_err = e
                failed.append(k)
        for k, fut in futs.items():
            try:
                results[k] = {
                    name: np.asarray(fut[i])
                    for i, name in enumerate(out_names)
                }
            except Exception as e:
                last_err = e
                failed.append(k)
        pending = failed
        if not pending:
            return results
        time.sleep(30.0 * (attempt + 1))
    raise RuntimeError(
        f"device execution failed after retries on cores {pending}"
    ) from last_err


def kernel(x, n_x, W, a):
    x = np.asarray(x, np.float32)
    n_x = np.asarray(n_x, np.float32)
    W = np.asarray(W, np.float32)
    a = np.asarray(a, np.float32)

    c = (a[SIZE_IN:, 0] @ W).astype(np.float32)          # (64,)
    nv = n_x.reshape(SIZE_IN, N_NEIGH)                    # raw reshape view
    ctile = np.ascontiguousarray(np.broadcast_to(c, (128, NI))).astype(np.float32)

    in_maps = []
    for k in range(N_CORES):
        j0 = SHARD * k
        ablk = np.zeros((128, NI, R), np.float32)
        blk = nv[:, j0 : j0 + SHARD]                      # (64, 62500) view
        ablk[:P] = blk.reshape(NI, P, R).transpose(1, 0, 2)
        bblk = np.zeros((128, R, NI), np.float32)
        bblk[:P] = n_x[j0 : j0 + SHARD].reshape(P, R, NI)
        in_maps.append({"ablk": ablk, "bblk": bblk, "cvec": ctile})

    res = _run_per_device(in_maps)

    pagg = np.zeros(SIZE_IN, np.float64)
    total = 0.0
    for k in range(N_CORES):
        po = np.asarray(res[k]["pout"], np.float64)       # (G, G*64)
        for m in range(G):
            pagg += po[m, SIZE_IN * m : SIZE_IN * (m + 1)]
        total += float(np.asarray(res[k]["usum"], np.float64).sum())

    agg = (pagg / total).reshape(SIZE_IN, 1)
    out = W.astype(np.float64) @ agg
    return (1.0 / (1.0 + np.exp(-out))).astype(np.float32)


# revision 13
# speedup vs baseline: 1.5896x; 1.5896x over previous
"""Distributed Trainium2 kernel for nn_AttentionMechanism (GNN message passing).

Reference math (single head GAT-style attention over N=500000 neighbors):
    nv   = n_x.reshape(64, N)          (raw row-major reshape, NOT a transpose)
    e    = a[:64].T @ (W@x) * 1s + (a[64:].T @ W) @ nv      (1, N)
    alpha= softmax(e)                   (softmax is shift-invariant -> the
                                         a[:64].(W@x) term cancels)
    out  = sigmoid(W @ (alpha @ n_x).T)

So with c = a[64:,0] @ W (a length-64 vector), alpha = softmax(c @ nv).
Energies are ~N(0,1) so exp() without max-subtraction is numerically safe.

Sharding: neighbor dim split across 8 cores (62500 each). Each core
computes its energies e_j = sum_i c_i * nv[i, j] (VectorE FMA over the
64 i-terms), u = exp(e) (ScalarE, with accum_out giving sum(u) free),
and the partial aggregate pagg = sum_j u_j * n_x[j, :] (TensorE,
contracting j over partitions in groups of 128). Host combines the 8
partial (pagg, sum_u) pairs, normalizes, applies W and sigmoid.
No device collectives needed.
"""

import numpy as np

from concourse import bacc, bass, bass2jax, mybir, tile

N_CORES = 8
SIZE_IN = 64
N_NEIGH = 500000
SHARD = N_NEIGH // N_CORES  # 62500
P = 125                     # used partitions (125 * 500 = 62500)
R = 500                     # neighbor rows per partition
NI = SIZE_IN                # 64
ICH = 8                     # i-values per A chunk (8 chunks of 8)
G = 5                       # r-columns per B matmul (rhs free = 5*64=320)
NB = R // G                 # 100 B matmuls
BCH = 10                    # B dma chunks
RB = R // BCH               # 50 rows per B dma chunk

F32 = mybir.dt.float32
BF16 = mybir.dt.bfloat16


def build_nc(n_devices=N_CORES):
    nc = bacc.Bacc(
        "TRN2",
        target_bir_lowering=False,
        debug=False,
        num_devices=n_devices,
    )
    a_ext = nc.dram_tensor("ablk", [128, NI, R], BF16, kind="ExternalInput")
    b_ext = nc.dram_tensor("bblk", [128, R, NI], BF16, kind="ExternalInput")
    c_ext = nc.dram_tensor("cvec", [128, NI], F32, kind="ExternalInput")
    pout_ext = nc.dram_tensor("pout", [G, G * NI], F32, kind="ExternalOutput")
    usum_ext = nc.dram_tensor("usum", [P, 1], F32, kind="ExternalOutput")

    with tile.TileContext(nc) as tc:
        with (
            tc.tile_pool(name="apool", bufs=2) as apool,
            tc.tile_pool(name="bpool", bufs=1) as bpool,
            tc.tile_pool(name="spool", bufs=1) as spool,
            tc.tile_pool(name="psum", bufs=1, space=bass.MemorySpace.PSUM) as pp,
        ):
            ctile = spool.tile([128, NI], F32)
            acc = spool.tile([128, R], F32)
            u = spool.tile([128, R], BF16)
            usum_sb = spool.tile([128, 1], F32)
            pout_sb = spool.tile([G, G * NI], F32)
            bsb = bpool.tile([128, R, NI], BF16)

            nc.sync.dma_start(ctile[:], c_ext[:])
            nc.vector.memset(acc[:], 0.0)
            nc.vector.memset(u[:], 0.0)

            # ---- A phase: energies e[p, r] = sum_i c_i * ablk[p, i, r] ----
            for ci in range(NI // ICH):
                at = apool.tile([128, ICH, R], BF16, tag="achunk")
                nc.sync.dma_start(
                    at[:], a_ext[:, ci * ICH : (ci + 1) * ICH, :]
                )
                for il in range(ICH):
                    i = ci * ICH + il
                    nc.vector.scalar_tensor_tensor(
                        out=acc[:],
                        in0=at[:, il, :],
                        scalar=ctile[:, i : i + 1],
                        in1=acc[:],
                        op0=mybir.AluOpType.mult,
                        op1=mybir.AluOpType.add,
                    )

            # ---- B dma: full-resident neighbor rows ----
            for cb in range(BCH):
                nc.sync.dma_start(
                    bsb[:, cb * RB : (cb + 1) * RB, :],
                    b_ext[:, cb * RB : (cb + 1) * RB, :],
                )

            # ---- exp + per-partition sums ----
            nc.scalar.activation(
                u[0:P, :],
                acc[0:P, :],
                mybir.ActivationFunctionType.Exp,
                accum_out=usum_sb[0:P, :],
            )

            # ---- B phase: pagg via PSUM-accumulated matmuls ----
            pacc = pp.tile([G, G * NI], F32)
            for T in range(NB):
                nc.tensor.matmul(
                    pacc[:],
                    u[:, G * T : G * (T + 1)],
                    bsb[:, G * T : G * (T + 1), :],
                    start=(T == 0),
                    stop=(T == NB - 1),
                )
            nc.vector.tensor_copy(pout_sb[:], pacc[:])

            nc.sync.dma_start(pout_ext[:], pout_sb[:])
            nc.sync.dma_start(usum_ext[:], usum_sb[0:P, :])

    nc.compile()
    return nc


_NC = None
_JITTED = None
_IO = None


def _get_nc():
    global _NC
    if _NC is None:
        _NC = build_nc()
    return _NC


def _get_runner():
    """Per-device async launcher.

    The stock ``run_bass_kernel_spmd`` axon path uses an 8-device
    ``shard_map``, which hangs in this container. Instead we jit the
    ``bass_exec`` primitive once and dispatch it on each NeuronCore
    with that core's shard (device-committed inputs), gathering all 8
    results afterwards — same NEFF, same per-core shapes, fully
    concurrent dispatch.
    """
    global _JITTED, _IO
    if _JITTED is not None:
        return _JITTED, _IO

    import jax

    nc = _get_nc()
    bass2jax.install_neuronx_cc_hook()
    assert nc.dbg_addr is None

    in_names = []
    out_names = []
    out_avals = []
    zero_outs = []
    for alloc in nc.m.functions[0].allocations:
        if not isinstance(alloc, mybir.MemoryLocationSet):
            continue
        name = alloc.memorylocations[0].name
        if alloc.kind == "ExternalInput":
            in_names.append(name)
        elif alloc.kind == "ExternalOutput":
            shape = tuple(alloc.tensor_shape)
            dtype = mybir.dt.np(alloc.dtype)
            out_names.append(name)
            out_avals.append(jax.core.ShapedArray(shape, dtype))
            zero_outs.append(np.zeros(shape, dtype))

    n_params = len(in_names)
    all_names = in_names + out_names
    donate = tuple(range(n_params, n_params + len(out_names)))

    def _body(*args):
        outs = bass2jax._bass_exec_p.bind(
            *args,
            out_avals=tuple(out_avals),
            in_names=tuple(all_names),
            out_names=tuple(out_names),
            lowering_input_output_aliases=(),
            sim_require_finite=True,
            sim_require_nnan=True,
            nc=nc,
        )
        return tuple(outs)

    _JITTED = jax.jit(_body, donate_argnums=donate, keep_unused=True)
    _IO = (in_names, out_names, zero_outs)
    return _JITTED, _IO


def _run_per_device(in_maps):
    import time

    import jax

    jitted, (in_names, out_names, zero_outs) = _get_runner()
    devs = jax.devices()[: len(in_maps)]

    def _dispatch(k):
        im = {**in_maps[k], "partition_id": np.array([[k]], np.uint32)}
        args = [jax.device_put(np.asarray(im[n]), devs[k]) for n in in_names]
        zs = [jax.device_put(z, devs[k]) for z in zero_outs]
        return jitted(*args, *zs)

    results = [None] * len(in_maps)
    pending = list(range(len(in_maps)))
    last_err = None
    for attempt in range(5):
        futs = {}
        failed = []
        for k in pending:
            try:
                futs[k] = _dispatch(k)
            except Exception as e:  # transient device errors; retry
                last_err = e
                failed.append(k)
        for k, fut in futs.items():
            try:
                results[k] = {
                    name: np.asarray(fut[i])
                    for i, name in enumerate(out_names)
                }
            except Exception as e:
                last_err = e
                failed.append(k)
        pending = failed
        if not pending:
            return results
        time.sleep(30.0 * (attempt + 1))
    raise RuntimeError(
        f"device execution failed after retries on cores {pending}"
    ) from last_err


def kernel(x, n_x, W, a):
    x = np.asarray(x, np.float32)
    n_x = np.asarray(n_x, np.float32)
    W = np.asarray(W, np.float32)
    a = np.asarray(a, np.float32)

    import ml_dtypes

    bf16 = ml_dtypes.bfloat16
    c = (a[SIZE_IN:, 0] @ W).astype(np.float32)          # (64,)
    nv = n_x.reshape(SIZE_IN, N_NEIGH)                    # raw reshape view
    ctile = np.ascontiguousarray(np.broadcast_to(c, (128, NI))).astype(np.float32)

    in_maps = []
    for k in range(N_CORES):
        j0 = SHARD * k
        ablk = np.zeros((128, NI, R), bf16)
        blk = nv[:, j0 : j0 + SHARD]                      # (64, 62500) view
        ablk[:P] = blk.reshape(NI, P, R).transpose(1, 0, 2).astype(bf16)
        bblk = np.zeros((128, R, NI), bf16)
        bblk[:P] = n_x[j0 : j0 + SHARD].reshape(P, R, NI).astype(bf16)
        in_maps.append({"ablk": ablk, "bblk": bblk, "cvec": ctile})

    res = _run_per_device(in_maps)

    pagg = np.zeros(SIZE_IN, np.float64)
    total = 0.0
    for k in range(N_CORES):
        po = np.asarray(res[k]["pout"], np.float64)       # (G, G*64)
        for m in range(G):
            pagg += po[m, SIZE_IN * m : SIZE_IN * (m + 1)]
        total += float(np.asarray(res[k]["usum"], np.float64).sum())

    agg = (pagg / total).reshape(SIZE_IN, 1)
    out = W.astype(np.float64) @ agg
    return (1.0 / (1.0 + np.exp(-out))).astype(np.float32)
